# revision 14
# baseline (speedup 1.0000x reference)
"""GroupedQueryAttention on 8 Trainium2 NeuronCores.

Sharding: core c = 4*b + g handles batch b (of 2) and KV group g (of 4),
i.e. 4 query heads (512 q-dims) + one 128-dim K/V head. o_proj is computed
as per-group partials (transposed layout); a per-t-chunk fp16 ReduceScatter
across the 4 cores of each batch runs overlapped with the next chunk's
compute, so the collective is almost entirely hidden. Core 4b+g ends up
with d-rows [512g, 512(g+1)) of out^T[b] for all t.

All matmuls run in fp16 (1 PE cycle/row) with fp32 PSUM accumulation.
Attention uses the transposed-AV formulation so V (not P) is the
stationary operand:
  - projections produce Q^T/K^T directly (lhsT=W tile, rhs=x^T tile)
  - scores are computed as S^T = (K^T).T @ Q^T           [s, t]
  - exp(S^T) = P^T feeds out^T = V^T @ P^T with lhsT=V (natural [s, hd]),
    one 512-row matmul per s-block instead of 4 LDWEIGHTS-bound 129-row
    matmuls, and the result lands directly in [hd, t] layout (no output
    transposes).
  - softmax denominators: P^T blocks are accumulated over s-blocks on the
    Vector engine, reduced across partitions with a single ones-vector
    matmul per head ([1,512] outputs packed into one PSUM bank at
    partition offsets 0/32/64/96), reciprocated on Vector, and
    partition-broadcast with a tiny SBUF->SBUF DMA.
  - o_proj for chunk tc-1 is interleaved into the s-loop of chunk tc to
    keep the tensor queue saturated while exp paces the s-loop.
"""

import math
import sys

import numpy as np

sys.path.insert(0, "/opt/trn_rl_repo")

B = 2
T = 2048
D = 2048
HEADS = 16
GROUPS = 4
HD = 128  # head dim
M = HEADS // GROUPS  # heads per group = 4
GQ = M * HD  # q dims per group = 512
SCALE = 1.0 / math.sqrt(HD)
N_CORES = 8
TCH = 512  # t chunk
NTCH = T // TCH  # 4
NSB = T // 128  # 16 s blocks
NKS = D // 128  # 16 contraction steps for projections

_COMPILED = {}


def _build():
    import concourse.bass as bass
    import concourse.mybir as mybir
    import concourse.tile as tile
    from concourse import bacc, bass_isa
    from concourse.masks import make_identity

    f16 = mybir.dt.float16
    f32 = mybir.dt.float32
    Exp = mybir.ActivationFunctionType.Exp
    Add = mybir.AluOpType.add
    Mult = mybir.AluOpType.mult

    nc = bacc.Bacc("TRN2", target_bir_lowering=False, num_devices=N_CORES)

    xT = nc.declare_dram_parameter("xT", [D, T], f16, isOutput=False)
    wq = nc.declare_dram_parameter("wq", [D, GQ], f16, isOutput=False)
    wk = nc.declare_dram_parameter("wk", [D, HD], f16, isOutput=False)
    wv = nc.declare_dram_parameter("wv", [D, HD], f16, isOutput=False)
    wo = nc.declare_dram_parameter("wo", [GQ, D], f16, isOutput=False)
    bqs_d = nc.declare_dram_parameter("bqs", [128, M], f32, isOutput=False)
    bks_d = nc.declare_dram_parameter("bks", [128, 1], f32, isOutput=False)
    bvs_d = nc.declare_dram_parameter("bvs", [128, 1], f32, isOutput=False)
    bo4_d = nc.declare_dram_parameter("bo4", [128, D // 128], f32, isOutput=False)
    outT = nc.declare_dram_parameter("outT", [TCH, T], f16, isOutput=True)

    groups = [[0, 1, 2, 3], [4, 5, 6, 7]]

    with tile.TileContext(nc) as tc:
        with (
            tc.tile_pool(name="const", bufs=1) as const,
            tc.tile_pool(name="work", bufs=2) as work,
            tc.tile_pool(name="psum", bufs=1, space="PSUM") as psum,
            tc.tile_pool(name="dram", bufs=1, space="DRAM") as dram,
        ):
            ident = const.tile([128, 128], f16)
            make_identity(nc, ident)
            bqs = const.tile([128, M], f32)
            bks = const.tile([128, 1], f32)
            bvs = const.tile([128, 1], f32)
            bo4 = const.tile([128, D // 128], f32)
            nc.sync.dma_start(bqs[:], bqs_d[:])
            nc.sync.dma_start(bks[:], bks_d[:])
            nc.sync.dma_start(bvs[:], bvs_d[:])
            nc.sync.dma_start(bo4[:], bo4_d[:])

            xt = const.tile([128, NKS, T], f16)
            wq_sb = const.tile([128, NKS, GQ], f16)
            wk_sb = const.tile([128, NKS, HD], f16)
            wv_sb = const.tile([128, NKS, HD], f16)
            wo_sb = const.tile([128, M, D], f16)
            # weights first so the JIT projections can start ASAP;
            # x chunks in tc-major order so the first chunk's contraction
            # inputs arrive before the whole 8MB of x.
            for i in range(NKS):
                nc.sync.dma_start(wk_sb[:, i, :], wk[i * 128 : (i + 1) * 128, :])
                nc.sync.dma_start(wv_sb[:, i, :], wv[i * 128 : (i + 1) * 128, :])
            for i in range(NKS):
                nc.sync.dma_start(wq_sb[:, i, :], wq[i * 128 : (i + 1) * 128, :])
            for tc_i in range(NTCH):
                tsl = slice(tc_i * TCH, (tc_i + 1) * TCH)
                for i in range(NKS):
                    nc.sync.dma_start(xt[:, i, tsl], xT[i * 128 : (i + 1) * 128, tsl])
            for h in range(M):
                nc.sync.dma_start(wo_sb[:, h, :], wo[h * 128 : (h + 1) * 128, :])

            qt = const.tile([128, M, T], f16)
            kt = const.tile([128, T], f16)
            vt_sb = const.tile([128, T], f16)
            v_sb = const.tile([128, NSB, HD], f16)

            # ---- K/V projection of one t-chunk + V transpose (emitted JIT
            # inside the first head's s-sweep) ----
            def kvproj_block(kv_i):
                tsl = slice(kv_i * TCH, (kv_i + 1) * TCH)
                acc = psum.tile([128, TCH], f32, tag="acc", bufs=3, name="acc")
                for ks in range(NKS):
                    nc.tensor.matmul(
                        acc[:],
                        wk_sb[:, ks, :],
                        xt[:, ks, tsl],
                        start=(ks == 0),
                        stop=(ks == NKS - 1),
                    )
                nc.vector.tensor_scalar_add(kt[:, tsl], acc[:], bks[:, 0:1])
                acc = psum.tile([128, TCH], f32, tag="acc", bufs=3, name="acc")
                for ks in range(NKS):
                    nc.tensor.matmul(
                        acc[:],
                        wv_sb[:, ks, :],
                        xt[:, ks, tsl],
                        start=(ks == 0),
                        stop=(ks == NKS - 1),
                    )
                nc.vector.tensor_scalar_add(vt_sb[:, tsl], acc[:], bvs[:, 0:1])
                for sb in range(4):
                    s = kv_i * 4 + sb
                    tp = psum.tile([128, 128], f16, tag="tp", bufs=1, name="tp")
                    nc.tensor.transpose(
                        tp[:], vt_sb[:, s * 128 : (s + 1) * 128], ident[:]
                    )
                    nc.vector.tensor_copy(v_sb[:, s, :], tp[:])

            # ---- per t-chunk: Q proj, attention, (interleaved) o_proj, RS ----
            at_prev = None
            partial_prev = None
            rs_jobs = []

            def oproj_block(cb, at_t, partials):
                pp = psum.tile([128, TCH], f32, tag="acc", bufs=3, name="pp")
                for h in range(M):
                    nc.tensor.matmul(
                        pp[:],
                        wo_sb[:, h, cb * 128 : (cb + 1) * 128],
                        at_t[:, h, :],
                        start=(h == 0),
                        stop=(h == M - 1),
                    )
                po = work.tile([128, TCH], f16, tag="po", bufs=4, name="po")
                nc.vector.tensor_scalar_add(po[:], pp[:], bo4[:, cb : cb + 1])
                half, cbh = divmod(cb, 8)
                nc.sync.dma_start(
                    partials[half][cbh * 128 : (cbh + 1) * 128, :], po[:]
                )

            for tc_i in range(NTCH):
                tsl = slice(tc_i * TCH, (tc_i + 1) * TCH)
                at_cur = work.tile([128, M, TCH], f16, tag="at", bufs=2, name="at")
                # head-outer attention: each head's partition-reduce /
                # reciprocal hides under the next head's s-sweep
                for h in range(M):
                    # JIT Q projection for this head
                    acc = psum.tile([128, TCH], f32, tag="acc", bufs=3, name="acc")
                    for ks in range(NKS):
                        nc.tensor.matmul(
                            acc[:],
                            wq_sb[:, ks, h * 128 : (h + 1) * 128],
                            xt[:, ks, tsl],
                            start=(ks == 0),
                            stop=(ks == NKS - 1),
                        )
                    nc.vector.tensor_scalar(
                        qt[:, h, tsl],
                        acc[:],
                        SCALE,
                        bqs[:, h : h + 1],
                        op0=Mult,
                        op1=Add,
                    )

                    den = work.tile([128, TCH], f16, tag="den", bufs=2, name="den")
                    ot = psum.tile([128, TCH], f32, tag="ot", bufs=3, name="ot")
                    for s in range(NSB):
                        # JIT K/V projections during the very first sweep
                        if tc_i == 0 and h == 0 and s % 4 == 0:
                            kvproj_block(s // 4)
                        p = work.tile([128, TCH], f16, tag="p", bufs=3, name="p")
                        sps = psum.tile([128, TCH], f32, tag="acc", bufs=3, name="sps")
                        nc.tensor.matmul(
                            sps[:],
                            kt[:, s * 128 : (s + 1) * 128],
                            qt[:, h, tsl],
                            start=True,
                            stop=True,
                        )
                        nc.scalar.activation(p[:], sps[:], Exp)
                        # denominator partial sums over s-blocks (Vector)
                        if s == 0:
                            nc.vector.tensor_copy(den[:], p[:])
                        else:
                            nc.vector.tensor_tensor(den[:], den[:], p[:], op=Add)
                        # transposed AV: out^T[hd, t] += V[s,:]^T @ P^T[s, t]
                        nc.tensor.matmul(
                            ot[:],
                            v_sb[:, s, :],
                            p[:],
                            start=(s == 0),
                            stop=(s == NSB - 1),
                        )
                        # interleave o_proj blocks of the previous chunk,
                        # packed into the first half so its ReduceScatter
                        # fires by mid-loop
                        it = h * NSB + s
                        if at_prev is not None and it % 2 == 0 and it < 32:
                            oproj_block(it // 2, at_prev, partial_prev)
                    # denominator: partition-reduce with broadcast output
                    # (gpsimd), fast reciprocal (Vector), normalize
                    denb = work.tile([128, TCH], f32, tag="denb", bufs=2, name="denb")
                    nc.gpsimd.partition_all_reduce(
                        denb[:], den[:], 128, bass_isa.ReduceOp.add
                    )
                    dbc = work.tile([128, TCH], f32, tag="dbc", bufs=2, name="dbc")
                    nc.vector.reciprocal_approx_fast(dbc[:], denb[:])
                    nc.vector.tensor_tensor(
                        at_cur[:, h, :], ot[:], dbc[:], op=Mult
                    )
                if at_prev is not None:
                    rs_jobs.append(partial_prev)

                partial_cur = [
                    dram.tile([D // 2, TCH], f16, tag=f"ptl{i}", bufs=2,
                              name=f"partial{i}")
                    for i in range(2)
                ]
                at_prev, partial_prev = at_cur, partial_cur

            # last chunk's o_proj runs un-interleaved
            for cb in range(D // 128):
                oproj_block(cb, at_prev, partial_prev)
            rs_jobs.append(partial_prev)

            # overlapped per-half-chunk fp16 ReduceScatter + output DMA
            for tc_i, partials in enumerate(rs_jobs):
                for half in range(2):
                    rs = dram.tile(
                        [TCH // 2, TCH], f16, tag="rso", bufs=4, name="rs"
                    )
                    nc.gpsimd.collective_compute(
                        "ReduceScatter",
                        mybir.AluOpType.add,
                        replica_groups=groups,
                        ins=[partials[half][:]],
                        outs=[rs[:]],
                    )
                    nc.sync.dma_start(
                        outT[
                            half * 256 : (half + 1) * 256,
                            tc_i * TCH : (tc_i + 1) * TCH,
                        ],
                        rs[:],
                    )

    nc.compile()
    return nc


def _get_nc():
    if "nc" not in _COMPILED:
        _COMPILED["nc"] = _build()
    return _COMPILED["nc"]


def kernel(x, Wq, bq, Wk, bk, Wv, bv, Wo, bo):
    from concourse.bass_utils import run_bass_kernel_spmd

    x = np.asarray(x, np.float32)
    Wq = np.asarray(Wq, np.float32)
    Wk = np.asarray(Wk, np.float32)
    Wv = np.asarray(Wv, np.float32)
    Wo = np.asarray(Wo, np.float32)
    bq = np.asarray(bq, np.float32)
    bk = np.asarray(bk, np.float32)
    bv = np.asarray(bv, np.float32)
    bo = np.asarray(bo, np.float32)

    nc = _get_nc()

    in_maps = []
    for c in range(N_CORES):
        b, g = c // 4, c % 4
        in_maps.append(
            {
                "xT": np.ascontiguousarray(x[b].T).astype(np.float16),
                "wq": np.ascontiguousarray(
                    Wq[:, g * GQ : (g + 1) * GQ]
                ).astype(np.float16),
                "wk": np.ascontiguousarray(
                    Wk[:, g * HD : (g + 1) * HD]
                ).astype(np.float16),
                "wv": np.ascontiguousarray(
                    Wv[:, g * HD : (g + 1) * HD]
                ).astype(np.float16),
                "wo": np.ascontiguousarray(
                    Wo[g * GQ : (g + 1) * GQ, :]
                ).astype(np.float16),
                "bqs": np.ascontiguousarray(
                    (bq[g * GQ : (g + 1) * GQ] * SCALE).reshape(M, 128).T
                ),
                "bks": np.ascontiguousarray(
                    bk[g * HD : (g + 1) * HD].reshape(1, 128).T
                ),
                "bvs": np.ascontiguousarray(
                    bv[g * HD : (g + 1) * HD].reshape(1, 128).T
                ),
                "bo4": np.ascontiguousarray((bo / 4.0).reshape(D // 128, 128).T),
            }
        )

    res = run_bass_kernel_spmd(nc, in_maps, list(range(N_CORES)))
    _COMPILED["last_res"] = res

    out = np.empty((B, T, D), np.float32)
    for b in range(B):
        for g in range(4):
            oT = res.results[4 * b + g]["outT"]  # [512, T] f16
            # upper half: d rows [256g, 256g+256); lower: 1024 + same
            out[b, :, 256 * g : 256 * g + 256] = oT[0:256].T.astype(np.float32)
            out[b, :, 1024 + 256 * g : 1024 + 256 * g + 256] = (
                oT[256:512].T.astype(np.float32)
            )
    return out
